# revision 28
# baseline (speedup 1.0000x reference)
"""Bass/Trainium2 kernel for nn_F_Loss_65446711656630.

Strategy (data-parallel over N, 8 cores):
  - Host: per core slice 8192 rows (64 chunks of 128 rows), lay them out
    rows-on-partitions in 16 four-chunk tiles: 11 tiles fp8e4m3 *bit-packed
    into uint32* ([128 x 512] u32 on the wire -> DMA descriptor cost on
    trn2 scales with element count, so packed fp8 moves at full physical
    bandwidth instead of the ~half-rate raw-fp8 descriptors get) + 5 tiles
    fp16 ([128 x 2048]); ~5.25 MiB per core vs 8.4 all-fp16.  Plus
    per-chunk one-hot class matrices W [128, 64*16] in fp16 and fp8.
    No sorting, no boundary fixups: the one-hot IS the segment assignment.
  - Device: fp8 tiles are bitcast back to [128 x 4096] fp8 views.
    Segment-sum as matmul on the TensorEngine: for each 128-row chunk k,
    psum[16, 512] += W_k^T @ chunk  gives per-class sums directly; a
    second accumulator takes W_k^T @ square(chunk).  Squares (fp16 out,
    for precision) are split ACT (1x) / DVE (1x fp8, 2x fp16) interleaved
    with arrival order so the square pass stays under the DMA stream
    rate.  Four PSUM accumulators
    (h/sq x even/odd chunks) live in four banks at column-group offsets
    0/32/64/96 so consecutive matmuls target different PE column groups
    and overlap.
  - Host: add the 4 accumulators (fp64), then the tiny O(C^2 D) pairwise
    betainc/top-k stage in f32 jax on CPU (mirroring the reference's
    numerics exactly).
"""

import numpy as np

C = 16
D = 512
N = 65536
NCORES = 8
ROWS = N // NCORES          # 8192 rows per core
P = 128                     # SBUF partitions / rows per chunk
CHUNKS = ROWS // P          # 64 chunks per core
TCH = 4                     # chunks per tile
NTILE = CHUNKS // TCH       # 16 tiles per core
TCOLS = TCH * D             # 2048 elements per tile per partition
XMIN, XMAX = 1e-37, 1.0 - 1e-5

# tile schedule in arrival order: "a" = fp8+ACT square, "d" = fp8+DVE,
# "f" = fp16+DVE (fp16 last: cheap 2x squares -> short tail).  GpSimd is
# kept off the square pass: its sw multiply contends with DVE for SBUF
# ports (measured: concurrent DVE multiplies ran 2.3x slower).
SCHED = ["a", "d", "a", "a", "d", "a", "a", "a",
         "d", "a", "a", "f", "f", "f", "f", "f"]
assert len(SCHED) == NTILE
FP8_TILES = [i for i, s in enumerate(SCHED) if s != "f"]
FP16_TILES = [i for i, s in enumerate(SCHED) if s == "f"]

_NC_CACHE = {}


def _build_nc():
    """Per-core SPMD program.

    Inputs:  "ht8" [11, 128, 512] uint32 (bit-packed fp8e4m3, arrival-order
                                          slot; fp8 col (c*512+f) of slot s
                                          = hidden[(4*tile+c)*128 + r, f])
             "htf" [5, 128, 2048] fp16   (fp16 tiles)
             "w16"/"w8" [128, 1024]      (w[r, k*16+q] = 1 iff row r of
                                          chunk k has class q)
    Output:  "ob" [128, 512] f32         (partitions 32g..32g+16 = group g:
                                          0 h-sums even chunks, 1 h-sums
                                          odd, 2 sq-sums even, 3 sq odd)
    """
    import concourse.tile as tile
    from concourse import bacc, mybir

    f32 = mybir.dt.float32
    f16 = mybir.dt.float16
    f8 = mybir.dt.float8e4
    u32 = mybir.dt.uint32

    nc = bacc.Bacc("TRN2", target_bir_lowering=False, debug=False,
                   num_devices=NCORES)
    ht8 = nc.declare_dram_parameter("ht8", [len(FP8_TILES), P, TCOLS // 4],
                                    u32, isOutput=False)
    htf = nc.declare_dram_parameter("htf", [len(FP16_TILES), P, TCOLS], f16,
                                    isOutput=False)
    w16 = nc.declare_dram_parameter("w16", [P, CHUNKS * C], f16,
                                    isOutput=False)
    w8 = nc.declare_dram_parameter("w8", [P, CHUNKS * C // 4], u32,
                                   isOutput=False)
    ob = nc.declare_dram_parameter("ob", [P, D], f32, isOutput=True)

    with tile.TileContext(nc) as tc:
        with (
            tc.tile_pool(name="p8", bufs=11) as pool8,
            tc.tile_pool(name="pf", bufs=5) as poolf,
            tc.tile_pool(name="sq", bufs=6) as sq_pool,
            tc.tile_pool(name="wp", bufs=1) as w_pool,
            tc.psum_pool(name="ps", bufs=1) as psum_pool,
        ):
            # hoist the ACT table load to the head of the program
            dummy = w_pool.tile([P, 8], f16, tag="dummy")
            nc.gpsimd.memset(dummy[:], 0)
            nc.scalar.square(dummy[:], dummy[:])

            # all tile DMAs issue up-front in arrival order from a single
            # engine (sync): per-queue FIFOs then complete tiles at issue
            # pace; pool capacities cover every tile so no issue ever
            # blocks on a consumption semaphore.  tile 0 goes before the
            # weight loads so the first square starts as early as possible.
            tiles = []
            s8 = sf = 0
            for i, kind in enumerate(SCHED):
                if kind == "f":
                    t = poolf.tile([P, TCOLS], f16, name=f"tf_{i}", tag="tf")
                    nc.sync.dma_start(t[:], htf[sf])
                    sf += 1
                    tiles.append(t[:])
                else:
                    t = pool8.tile([P, TCOLS // 4], u32, name=f"t8_{i}",
                                   tag="t8")
                    nc.sync.dma_start(t[:], ht8[s8])
                    s8 += 1
                    tiles.append(t[:].bitcast(f8))
                if i == 0:
                    wsb16 = w_pool.tile([P, CHUNKS * C], f16, tag="wsb16")
                    nc.sync.dma_start(wsb16[:], w16[:])
                    wsb8_u32 = w_pool.tile([P, CHUNKS * C // 4], u32,
                                           tag="wsb8_u32")
                    nc.sync.dma_start(wsb8_u32[:], w8[:])
                    wsb8 = wsb8_u32[:].bitcast(f8)

            banks = [psum_pool.tile([P, D], f32, name=f"acc{g}",
                                    tag=f"acc{g}")
                     for g in range(4)]
            accs = [banks[g][32 * g:32 * g + C, :] for g in range(4)]
            started = [False] * 4
            n_issued = [0] * 4

            def seg_mm(g, k, wsb, rhs):
                first = not started[g]
                started[g] = True
                n_issued[g] += 1
                last = n_issued[g] == CHUNKS // 2
                nc.tensor.matmul(
                    accs[g], wsb[:, k * C:(k + 1) * C], rhs,
                    start=first, stop=last, tile_position=(0, 32 * g))

            sq_done = []   # (sq tile, tile index)

            def issue_sq_mms():
                while sq_done:
                    sq, i = sq_done.pop(0)
                    for c in range(TCH):
                        k = i * TCH + c
                        seg_mm(2 + (k % 2), k, wsb16,
                               sq[:, c * D:(c + 1) * D])

            for i, kind in enumerate(SCHED):
                tv = tiles[i]
                hw = wsb16 if kind == "f" else wsb8

                sq = sq_pool.tile([P, TCOLS], f16, name=f"sq_{i}", tag="sq")
                if kind == "a":
                    nc.scalar.square(sq[:], tv)
                else:
                    nc.vector.tensor_mul(sq[:], tv, tv)

                # h matmuls for this tile; sq matmuls for finished tiles
                for c in range(TCH):
                    k = i * TCH + c
                    seg_mm(k % 2, k, hw, tv[:, c * D:(c + 1) * D])
                issue_sq_mms()
                sq_done.append((sq, i))
            issue_sq_mms()

            ob_sb = w_pool.tile([P, D], f32, tag="ob_sb")
            for g in range(4):
                if g % 2 == 0:
                    nc.vector.tensor_copy(ob_sb[32 * g:32 * g + C, :], accs[g])
                else:
                    nc.scalar.copy(ob_sb[32 * g:32 * g + C, :], accs[g])
            # h-sum half can ship while the sq copies still run
            nc.sync.dma_start(ob[0:64], ob_sb[0:64])
            nc.sync.dma_start(ob[64:128], ob_sb[64:128])
    nc.compile()
    return nc


def _get_nc():
    if "nc" not in _NC_CACHE:
        _NC_CACHE["nc"] = _build_nc()
    return _NC_CACHE["nc"]


def _prep_core(h_k, ids_k):
    import ml_dtypes

    ch = h_k.reshape(CHUNKS, P, D)
    T8 = np.empty((len(FP8_TILES), P, TCOLS // 4), dtype=np.uint32)
    Tf = np.empty((len(FP16_TILES), P, TCOLS), dtype=np.float16)
    s8 = sf = 0
    for i, kind in enumerate(SCHED):
        blk = ch[i * TCH:(i + 1) * TCH].transpose(1, 0, 2).reshape(P, TCOLS)
        if kind == "f":
            Tf[sf] = blk.astype(np.float16)
            sf += 1
        else:
            T8[s8] = np.ascontiguousarray(
                blk.astype(ml_dtypes.float8_e4m3)).view(np.uint32)
            s8 += 1

    ids2 = ids_k.reshape(CHUNKS, P)
    W3 = np.zeros((P, CHUNKS, C), dtype=np.float16)
    k_idx = np.broadcast_to(np.arange(CHUNKS)[:, None], (CHUNKS, P))
    r_idx = np.broadcast_to(np.arange(P)[None, :], (CHUNKS, P))
    W3[r_idx, k_idx, ids2] = 1.0
    W16 = W3.reshape(P, CHUNKS * C)
    W8 = np.ascontiguousarray(
        W16.astype(ml_dtypes.float8_e4m3)).view(np.uint32)
    return T8, Tf, W16, W8


def _device_stats(hidden, ids, **run_kwargs):
    """Returns (sums[C,D], sumsq[C,D]) float64, plus the raw run result."""
    from concourse import bass_utils

    nc = _get_nc()

    in_maps = []
    for k in range(NCORES):
        rows = slice(k * ROWS, (k + 1) * ROWS)
        T8, Tf, W16, W8 = _prep_core(hidden[rows], ids[rows])
        in_maps.append({"ht8": T8, "htf": Tf, "w16": W16, "w8": W8})

    res = bass_utils.run_bass_kernel_spmd(nc, in_maps, list(range(NCORES)),
                                          **run_kwargs)

    sums = np.zeros((C, D), dtype=np.float64)
    sumsq = np.zeros((C, D), dtype=np.float64)
    for k in range(NCORES):
        ob = res.results[k]["ob"].astype(np.float64)
        sums += ob[0:C] + ob[32:32 + C]
        sumsq += ob[64:64 + C] + ob[96:96 + C]
    return sums, sumsq, res


def _pairwise_loss(counts, sums, sumsq, d):
    """The tiny O(C^2 D) stage on host CPU.

    Runs in float32 with the same jax ops as the reference: at these extreme
    betainc parameters (b ~ 8190, x ~ 1e-5) jax's f32 betainc differs from
    the true (f64) value by ~1e-3, so matching the reference requires
    replicating its f32 numerics, not improving on them.
    """
    import jax
    import jax.numpy as jnp

    cpu = jax.devices("cpu")[0]
    with jax.default_device(cpu):
        counts64 = counts.astype(np.float64)
        means64 = sums / counts64[:, None]
        withins64 = sumsq - counts64[:, None] * means64**2
        counts = jnp.asarray(counts64, jnp.float32)               # [C]
        means = jnp.asarray(means64, jnp.float32)                 # [C, D]
        withins = jnp.asarray(withins64, jnp.float32)             # [C, D]
        half_diff = (means[:, None, :] - means[None, :, :]) * 0.5
        pair_counts = counts[:, None] + counts[None, :]
        pair_between = half_diff * half_diff * pair_counts[:, :, None]
        pair_within = withins[:, None, :] + withins[None, :, :]
        d2 = pair_counts - 2.0
        d2 = jnp.where(d2 == 0.0, 1e-5, d2)
        x = pair_between / (pair_between + pair_within)
        x = jnp.clip(x, XMIN, XMAX)
        a = jnp.full_like(x, 0.5)
        b = jnp.broadcast_to((d2 * 0.5)[:, :, None], x.shape)
        xbetainc = jax.scipy.special.betainc(a, b, x)             # [C, C, D]
        top_k, _ = jax.lax.top_k(xbetainc, int(d))                # [C, C, d]
        per_pair = jnp.sum(jnp.log(top_k), axis=-1)               # [C, C]
        mask = jnp.triu(jnp.ones((C, C), dtype=bool), k=1)
        total = jnp.sum(jnp.where(mask, per_pair, jnp.zeros_like(per_pair)))
        return float(-total)


def kernel(hidden, batch_ids, d):
    hidden = np.asarray(hidden, dtype=np.float32)
    ids = np.asarray(batch_ids).astype(np.int64)
    assert hidden.shape == (N, D), hidden.shape

    counts = np.bincount(ids, minlength=C).astype(np.float64)
    sums, sumsq, _ = _device_stats(hidden, ids)
    total = _pairwise_loss(counts, sums, sumsq, int(np.asarray(d)))
    return np.array(total, dtype=np.float32)


# revision 29
# speedup vs baseline: 1.0936x; 1.0936x over previous
"""Bass/Trainium2 kernel for nn_F_Loss_65446711656630.

Strategy (data-parallel over N, 8 cores):
  - Host: per core slice 8192 rows (64 chunks of 128 rows), lay them out
    rows-on-partitions in 16 four-chunk tiles: 11 tiles fp8e4m3 *bit-packed
    into uint32* ([128 x 512] u32 on the wire -> DMA descriptor cost on
    trn2 scales with element count, so packed fp8 moves at full physical
    bandwidth instead of the ~half-rate raw-fp8 descriptors get) + 5 tiles
    fp16 ([128 x 2048]); ~5.25 MiB per core vs 8.4 all-fp16.  Plus
    per-chunk one-hot class matrices W [128, 64*16] in fp16 and fp8.
    No sorting, no boundary fixups: the one-hot IS the segment assignment.
  - Device: fp8 tiles are bitcast back to [128 x 4096] fp8 views.
    Segment-sum as matmul on the TensorEngine: for each 128-row chunk k,
    psum[16, 512] += W_k^T @ chunk  gives per-class sums directly; a
    second accumulator takes W_k^T @ square(chunk).  Squares (fp16 out,
    for precision) are split ACT (1x) / DVE (1x fp8, 2x fp16) interleaved
    with arrival order so the square pass stays under the DMA stream
    rate.  Four PSUM accumulators
    (h/sq x even/odd chunks) live in four banks at column-group offsets
    0/32/64/96 so consecutive matmuls target different PE column groups
    and overlap.
  - Host: add the 4 accumulators (fp64), then the tiny O(C^2 D) pairwise
    betainc/top-k stage in f32 jax on CPU (mirroring the reference's
    numerics exactly).
"""

import numpy as np

C = 16
D = 512
N = 65536
NCORES = 8
ROWS = N // NCORES          # 8192 rows per core
P = 128                     # SBUF partitions / rows per chunk
CHUNKS = ROWS // P          # 64 chunks per core
TCH = 4                     # chunks per tile
NTILE = CHUNKS // TCH       # 16 tiles per core
TCOLS = TCH * D             # 2048 elements per tile per partition
XMIN, XMAX = 1e-37, 1.0 - 1e-5

# tile schedule in arrival order: "a" = fp8+ACT square, "d" = fp8+DVE,
# "f" = fp16+DVE (fp16 last: cheap 2x squares -> short tail).  GpSimd is
# kept off the square pass: its sw multiply contends with DVE for SBUF
# ports (measured: concurrent DVE multiplies ran 2.3x slower).
SCHED = ["a", "d", "a", "d", "a", "a", "d", "a",
         "a", "d", "a", "f", "f", "f", "f", "f"]
assert len(SCHED) == NTILE
FP8_TILES = [i for i, s in enumerate(SCHED) if s != "f"]
FP16_TILES = [i for i, s in enumerate(SCHED) if s == "f"]

_NC_CACHE = {}


def _build_nc():
    """Per-core SPMD program.

    Inputs:  "ht8" [11, 128, 512] uint32 (bit-packed fp8e4m3, arrival-order
                                          slot; fp8 col (c*512+f) of slot s
                                          = hidden[(4*tile+c)*128 + r, f])
             "htf" [5, 128, 2048] fp16   (fp16 tiles)
             "w16"/"w8" [128, 1024]      (w[r, k*16+q] = 1 iff row r of
                                          chunk k has class q)
    Output:  "ob" [128, 512] f32         (partitions 32g..32g+16 = group g:
                                          0 h-sums even chunks, 1 h-sums
                                          odd, 2 sq-sums even, 3 sq odd)
    """
    import concourse.tile as tile
    from concourse import bacc, mybir

    f32 = mybir.dt.float32
    f16 = mybir.dt.float16
    f8 = mybir.dt.float8e4
    u32 = mybir.dt.uint32

    nc = bacc.Bacc("TRN2", target_bir_lowering=False, debug=False,
                   num_devices=NCORES)
    ht8 = nc.declare_dram_parameter("ht8", [len(FP8_TILES), P, TCOLS // 4],
                                    u32, isOutput=False)
    htf = nc.declare_dram_parameter("htf", [len(FP16_TILES), P, TCOLS], f16,
                                    isOutput=False)
    w16 = nc.declare_dram_parameter("w16", [P, CHUNKS * C], f16,
                                    isOutput=False)
    w8 = nc.declare_dram_parameter("w8", [P, CHUNKS * C // 4], u32,
                                   isOutput=False)
    ob = nc.declare_dram_parameter("ob", [P, D], f32, isOutput=True)

    with tile.TileContext(nc) as tc:
        with (
            tc.tile_pool(name="p8", bufs=11) as pool8,
            tc.tile_pool(name="pf", bufs=5) as poolf,
            tc.tile_pool(name="sq", bufs=6) as sq_pool,
            tc.tile_pool(name="wp", bufs=1) as w_pool,
            tc.psum_pool(name="ps", bufs=1) as psum_pool,
        ):
            # hoist the ACT table load to the head of the program
            dummy = w_pool.tile([P, 8], f16, tag="dummy")
            nc.gpsimd.memset(dummy[:], 0)
            nc.scalar.square(dummy[:], dummy[:])

            # all tile DMAs issue up-front in arrival order from a single
            # engine (sync): per-queue FIFOs then complete tiles at issue
            # pace; pool capacities cover every tile so no issue ever
            # blocks on a consumption semaphore.  tile 0 goes before the
            # weight loads so the first square starts as early as possible.
            tiles = []
            s8 = sf = 0
            for i, kind in enumerate(SCHED):
                if kind == "f":
                    t = poolf.tile([P, TCOLS], f16, name=f"tf_{i}", tag="tf")
                    nc.sync.dma_start(t[:], htf[sf])
                    sf += 1
                    tiles.append(t[:])
                else:
                    t = pool8.tile([P, TCOLS // 4], u32, name=f"t8_{i}",
                                   tag="t8")
                    nc.sync.dma_start(t[:], ht8[s8])
                    s8 += 1
                    tiles.append(t[:].bitcast(f8))
                if i == 0:
                    wsb16 = w_pool.tile([P, CHUNKS * C], f16, tag="wsb16")
                    nc.sync.dma_start(wsb16[:], w16[:])
                    wsb8_u32 = w_pool.tile([P, CHUNKS * C // 4], u32,
                                           tag="wsb8_u32")
                    nc.sync.dma_start(wsb8_u32[:], w8[:])
                    wsb8 = wsb8_u32[:].bitcast(f8)

            banks = [psum_pool.tile([P, D], f32, name=f"acc{g}",
                                    tag=f"acc{g}")
                     for g in range(4)]
            accs = [banks[g][32 * g:32 * g + C, :] for g in range(4)]
            started = [False] * 4
            n_issued = [0] * 4

            def seg_mm(g, k, wsb, rhs):
                first = not started[g]
                started[g] = True
                n_issued[g] += 1
                last = n_issued[g] == CHUNKS // 2
                nc.tensor.matmul(
                    accs[g], wsb[:, k * C:(k + 1) * C], rhs,
                    start=first, stop=last, tile_position=(0, 32 * g))

            sq_done = []   # (sq tile, tile index)

            def issue_sq_mms():
                while sq_done:
                    sq, i = sq_done.pop(0)
                    for c in range(TCH):
                        k = i * TCH + c
                        seg_mm(2 + (k % 2), k, wsb16,
                               sq[:, c * D:(c + 1) * D])

            for i, kind in enumerate(SCHED):
                tv = tiles[i]
                hw = wsb16 if kind == "f" else wsb8

                sq = sq_pool.tile([P, TCOLS], f16, name=f"sq_{i}", tag="sq")
                if kind == "a":
                    nc.scalar.square(sq[:], tv)
                else:
                    nc.vector.tensor_mul(sq[:], tv, tv)

                # h matmuls for this tile; sq matmuls for finished tiles
                for c in range(TCH):
                    k = i * TCH + c
                    seg_mm(k % 2, k, hw, tv[:, c * D:(c + 1) * D])
                issue_sq_mms()
                sq_done.append((sq, i))
            issue_sq_mms()

            ob_sb = w_pool.tile([P, D], f32, tag="ob_sb")
            for g in range(4):
                if g % 2 == 0:
                    nc.vector.tensor_copy(ob_sb[32 * g:32 * g + C, :], accs[g])
                else:
                    nc.scalar.copy(ob_sb[32 * g:32 * g + C, :], accs[g])
            # h-sum half can ship while the sq copies still run
            nc.sync.dma_start(ob[0:64], ob_sb[0:64])
            nc.sync.dma_start(ob[64:128], ob_sb[64:128])
    nc.compile()
    return nc


def _get_nc():
    if "nc" not in _NC_CACHE:
        _NC_CACHE["nc"] = _build_nc()
    return _NC_CACHE["nc"]


def _prep_core(h_k, ids_k):
    import ml_dtypes

    ch = h_k.reshape(CHUNKS, P, D)
    T8 = np.empty((len(FP8_TILES), P, TCOLS // 4), dtype=np.uint32)
    Tf = np.empty((len(FP16_TILES), P, TCOLS), dtype=np.float16)
    s8 = sf = 0
    for i, kind in enumerate(SCHED):
        blk = ch[i * TCH:(i + 1) * TCH].transpose(1, 0, 2).reshape(P, TCOLS)
        if kind == "f":
            Tf[sf] = blk.astype(np.float16)
            sf += 1
        else:
            T8[s8] = np.ascontiguousarray(
                blk.astype(ml_dtypes.float8_e4m3)).view(np.uint32)
            s8 += 1

    ids2 = ids_k.reshape(CHUNKS, P)
    W3 = np.zeros((P, CHUNKS, C), dtype=np.float16)
    k_idx = np.broadcast_to(np.arange(CHUNKS)[:, None], (CHUNKS, P))
    r_idx = np.broadcast_to(np.arange(P)[None, :], (CHUNKS, P))
    W3[r_idx, k_idx, ids2] = 1.0
    W16 = W3.reshape(P, CHUNKS * C)
    W8 = np.ascontiguousarray(
        W16.astype(ml_dtypes.float8_e4m3)).view(np.uint32)
    return T8, Tf, W16, W8


def _device_stats(hidden, ids, **run_kwargs):
    """Returns (sums[C,D], sumsq[C,D]) float64, plus the raw run result."""
    from concourse import bass_utils

    nc = _get_nc()

    in_maps = []
    for k in range(NCORES):
        rows = slice(k * ROWS, (k + 1) * ROWS)
        T8, Tf, W16, W8 = _prep_core(hidden[rows], ids[rows])
        in_maps.append({"ht8": T8, "htf": Tf, "w16": W16, "w8": W8})

    res = bass_utils.run_bass_kernel_spmd(nc, in_maps, list(range(NCORES)),
                                          **run_kwargs)

    sums = np.zeros((C, D), dtype=np.float64)
    sumsq = np.zeros((C, D), dtype=np.float64)
    for k in range(NCORES):
        ob = res.results[k]["ob"].astype(np.float64)
        sums += ob[0:C] + ob[32:32 + C]
        sumsq += ob[64:64 + C] + ob[96:96 + C]
    return sums, sumsq, res


def _pairwise_loss(counts, sums, sumsq, d):
    """The tiny O(C^2 D) stage on host CPU.

    Runs in float32 with the same jax ops as the reference: at these extreme
    betainc parameters (b ~ 8190, x ~ 1e-5) jax's f32 betainc differs from
    the true (f64) value by ~1e-3, so matching the reference requires
    replicating its f32 numerics, not improving on them.
    """
    import jax
    import jax.numpy as jnp

    cpu = jax.devices("cpu")[0]
    with jax.default_device(cpu):
        counts64 = counts.astype(np.float64)
        means64 = sums / counts64[:, None]
        withins64 = sumsq - counts64[:, None] * means64**2
        counts = jnp.asarray(counts64, jnp.float32)               # [C]
        means = jnp.asarray(means64, jnp.float32)                 # [C, D]
        withins = jnp.asarray(withins64, jnp.float32)             # [C, D]
        half_diff = (means[:, None, :] - means[None, :, :]) * 0.5
        pair_counts = counts[:, None] + counts[None, :]
        pair_between = half_diff * half_diff * pair_counts[:, :, None]
        pair_within = withins[:, None, :] + withins[None, :, :]
        d2 = pair_counts - 2.0
        d2 = jnp.where(d2 == 0.0, 1e-5, d2)
        x = pair_between / (pair_between + pair_within)
        x = jnp.clip(x, XMIN, XMAX)
        a = jnp.full_like(x, 0.5)
        b = jnp.broadcast_to((d2 * 0.5)[:, :, None], x.shape)
        xbetainc = jax.scipy.special.betainc(a, b, x)             # [C, C, D]
        top_k, _ = jax.lax.top_k(xbetainc, int(d))                # [C, C, d]
        per_pair = jnp.sum(jnp.log(top_k), axis=-1)               # [C, C]
        mask = jnp.triu(jnp.ones((C, C), dtype=bool), k=1)
        total = jnp.sum(jnp.where(mask, per_pair, jnp.zeros_like(per_pair)))
        return float(-total)


def kernel(hidden, batch_ids, d):
    hidden = np.asarray(hidden, dtype=np.float32)
    ids = np.asarray(batch_ids).astype(np.int64)
    assert hidden.shape == (N, D), hidden.shape

    counts = np.bincount(ids, minlength=C).astype(np.float64)
    sums, sumsq, _ = _device_stats(hidden, ids)
    total = _pairwise_loss(counts, sums, sumsq, int(np.asarray(d)))
    return np.array(total, dtype=np.float32)
